# revision 11
# baseline (speedup 1.0000x reference)
"""AttentionMIL on 8 TRN2 NeuronCores (Bass/Tile), data-parallel by bags,
with all GEMMs in fp8(e4m3) DoubleRow mode (2 k-subtiles contracted per
pass -> ~2x TensorE throughput vs bf16).

Model (per reference):
    h  = relu(relu(X @ W1 + b1) @ W2 + b2)            # [N, 512]
    s  = tanh(h @ Wa1 + ba1) @ Wa2 + ba2              # [N]
    w  = segment_softmax(s, bags)                     # per-bag softmax
    bag_emb = segment_sum(w * h)                      # [B, 512]
    out = bag_emb @ Wc + bc                           # [B, 2]

Distribution: 32 bags -> 8 cores x 4 bags, device-local reductions, no
collectives.  Each core's patches are laid out with every bag padded to a
multiple of SUBT(=256), the whole core padded to a multiple of CHUNK
(=1024), so every 256-patch pooling subtile lies in exactly one bag.

The whole chain runs in transposed space (features on SBUF partitions,
patches on the free axis); weights are the stationary operands.  X is
pre-transposed and cast to fp8 on the host: xb[p, kk, n] = X[n, 128*kk+p].

fp8 scaling (e4m3: max 240, min normal 2^-6):
    w1 = 4*W1, w2 = 2*W2, wa1 = 2*Wa1, wa2 = 16*Wa2 (all fp8)
    ps1 = X8 @ (4 W1)         -> h1 := fp8(relu(ps1 + 4 b1) - mu) (DVE)
    ps2 = h1 @ (2 W2)         -> h28 := fp8(relu(ps2 + bias2))  = 8*h2
    psa = (8h2) @ (2 Wa1)     -> aT := fp8(tanh(psa/16 + ba1))
    psr = aT @ (16 Wa2) + pen -> e := bf16(exp(psr/16))
mu is an E[4*h1] estimate from a host-side subsample; centering h1 kills
the coherent E[h1] @ (W2 - fp8(W2)) quantization error, and the exact
mu @ W2 term is folded into bias2 = 8*b2 + 2*mu@W2.  pen = -25600 on
padding rows (so e == 0 there; replaces a mask; ba2 is dropped -- softmax
is shift-invariant).  Per-bag max subtraction is dropped: tanh bounds
|s| < 17 so exp cannot overflow.

Pooling: e broadcast across partitions via GpSimd partition_broadcast,
then per 256-patch subtile one fused VectorE scalar_tensor_tensor
(product 8*e*h2 with free-axis accumulate) into subs[p, m, t].  The host
sums each bag's subtile columns (f64), divides by 8*sum(e), and applies
the tiny classifier: out = bag_emb @ Wc + bc.
"""

import numpy as np
import ml_dtypes

import concourse.tile as tile
from concourse import bacc, mybir
from concourse.bass_utils import run_bass_kernel_spmd

N_CORES = 8
N_BAGS = 32
BAGS_PER_CORE = N_BAGS // N_CORES  # 4
D_IN = 1024
D_FEAT = 512
D_ATTN = 256
CHUNK = 1024  # patches per pipeline chunk
SUBT = 256    # pooling subtile (bag padding granularity)
NMM = 512     # matmul moving free dim

BF16 = mybir.dt.bfloat16
F32 = mybir.dt.float32
FP8 = mybir.dt.float8e4
DR = mybir.MatmulPerfMode.DoubleRow
NP_FP8 = ml_dtypes.float8_e4m3  # TRN FP8_EXP4-compatible
NP_BF16 = ml_dtypes.bfloat16

_build_cache: dict = {}

V_REPEAT = 1  # For_i repetitions of the whole body (timing harness only)


def _build(np_pad: int) -> "bacc.Bacc":
    """Build + compile the per-core program for NP_PAD padded patches."""
    key = (np_pad, V_REPEAT)
    if key in _build_cache:
        return _build_cache[key]

    assert np_pad % CHUNK == 0
    C = np_pad // CHUNK          # number of chunks
    T = np_pad // SUBT           # number of pooling subtiles
    KI = D_IN // 128             # 8  k-subtiles for GEMM1
    MF = D_FEAT // 128           # 4  feature subtiles
    MA = D_ATTN // 128           # 2  attention subtiles
    SUBS = CHUNK // SUBT         # 4  pooling subtiles per chunk
    NH = CHUNK // NMM            # 2  matmul n-halves per chunk

    nc = bacc.Bacc("TRN2", target_bir_lowering=False, debug=False)

    xb = nc.dram_tensor("xb", [128, KI, np_pad], FP8, kind="ExternalInput").ap()
    w1 = nc.dram_tensor("w1", [128, KI, D_FEAT], FP8, kind="ExternalInput").ap()
    w2 = nc.dram_tensor("w2", [128, MF, D_FEAT], FP8, kind="ExternalInput").ap()
    wa1 = nc.dram_tensor("wa1", [128, MF, D_ATTN], FP8, kind="ExternalInput").ap()
    wa2 = nc.dram_tensor("wa2", [128, MA, 16], FP8, kind="ExternalInput").ap()
    nb1 = nc.dram_tensor("nb1", [128, MF], F32, kind="ExternalInput").ap()
    a1 = nc.dram_tensor("a1", [128, MF], F32, kind="ExternalInput").ap()
    b2 = nc.dram_tensor("b2", [128, MF], F32, kind="ExternalInput").ap()
    ba1 = nc.dram_tensor("ba1", [128, MA], F32, kind="ExternalInput").ap()
    pen = nc.dram_tensor("pen", [1, np_pad], BF16, kind="ExternalInput").ap()
    one16 = nc.dram_tensor("one16", [1, 16], BF16, kind="ExternalInput").ap()

    # per-subtile pooled sums: subs[p, m, t] = sum_{n in subtile t} 8*e*h2
    subs_out = nc.dram_tensor(
        "subs", [128, MF, T], F32, kind="ExternalOutput"
    ).ap()
    # e values (bf16, exactly as used in the pooling), padding rows -> 0
    e_out = nc.dram_tensor("e_out", [1, np_pad], BF16, kind="ExternalOutput").ap()

    AF = mybir.ActivationFunctionType
    OP = mybir.AluOpType

    import contextlib

    with tile.TileContext(nc) as tc:
        with (
            tc.tile_pool(name="const", bufs=1) as const,
            tc.tile_pool(name="xT", bufs=3) as xT_pool,
            tc.tile_pool(name="h1T", bufs=2) as h1T_pool,
            tc.tile_pool(name="h28", bufs=2) as h28_pool,
            tc.tile_pool(name="aT", bufs=2) as aT_pool,
            tc.tile_pool(name="erow", bufs=3) as e_pool,
            tc.tile_pool(name="ebc", bufs=2) as eb_pool,
            tc.tile_pool(name="dump", bufs=2) as dump_pool,
            tc.tile_pool(name="psw", bufs=3, space="PSUM") as psum_work,
            tc.tile_pool(name="psr", bufs=1, space="PSUM") as psum_r_pool,
        ):
            # ---- load constants ----
            w1_sb = const.tile([128, KI, D_FEAT], FP8)
            nc.sync.dma_start(w1_sb[:], w1[:])
            nb1_sb = const.tile([128, MF], F32)
            nc.sync.dma_start(nb1_sb[:], nb1[:])
            a1_sb = const.tile([128, MF], F32)
            nc.sync.dma_start(a1_sb[:], a1[:])
            w2_sb = const.tile([128, MF, D_FEAT], FP8)
            nc.sync.dma_start(w2_sb[:], w2[:])
            wa1_sb = const.tile([128, MF, D_ATTN], FP8)
            nc.sync.dma_start(wa1_sb[:], wa1[:])
            wa2_sb = const.tile([128, MA, 16], FP8)
            nc.sync.dma_start(wa2_sb[:], wa2[:])
            b2_sb = const.tile([128, MF], F32)
            nc.sync.dma_start(b2_sb[:], b2[:])
            ba1_sb = const.tile([128, MA], F32)
            nc.sync.dma_start(ba1_sb[:], ba1[:])
            pen_sb = const.tile([1, np_pad], BF16)
            nc.sync.dma_start(pen_sb[:], pen[:])
            one_sb = const.tile([1, 16], BF16)
            nc.sync.dma_start(one_sb[:], one16[:])

            subs_sb = const.tile([128, MF, T], F32)

            def emit_xT(cc):
                xt = xT_pool.tile([128, KI, CHUNK], FP8)
                nc.sync.dma_start(
                    xt[:], xb[:, :, cc * CHUNK : (cc + 1) * CHUNK]
                )
                return xt

            def chunk_body(c, xts):
                np0 = c * CHUNK

                xT = xts.pop(c)
                if c + 2 < C:
                    xts[c + 2] = emit_xT(c + 2)

                # ---- GEMM1: h1T = fp8(relu(W1^T X^T + 4 b1) - mu), DVE:
                # relu(x+b)-m = max(x, -b) + (b-m) -> one tensor_scalar ----
                h1T = h1T_pool.tile([128, MF, CHUNK], FP8)
                for m in range(MF):
                    ps = psum_work.tile([128, CHUNK], F32, tag="ps")
                    for nh in range(NH):
                        nsl = slice(nh * NMM, (nh + 1) * NMM)
                        for kp in range(KI // 2):
                            nc.tensor.matmul(
                                ps[:, nsl],
                                w1_sb[:, 2 * kp : 2 * kp + 2, m * 128 : (m + 1) * 128],
                                xT[:, 2 * kp : 2 * kp + 2, nsl],
                                start=(kp == 0),
                                stop=(kp == KI // 2 - 1),
                                perf_mode=DR,
                            )
                    nc.vector.tensor_scalar(
                        h1T[:, m, :], ps[:], nb1_sb[:, m : m + 1],
                        a1_sb[:, m : m + 1], OP.max, OP.add,
                    )

                # ---- GEMM2: h28 = fp8(relu(. + bias2)) on ScalarE ----
                h28 = h28_pool.tile([128, MF, CHUNK], FP8)
                for m in range(MF):
                    ps = psum_work.tile([128, CHUNK], F32, tag="ps")
                    for nh in range(NH):
                        nsl = slice(nh * NMM, (nh + 1) * NMM)
                        for kp in range(MF // 2):
                            nc.tensor.matmul(
                                ps[:, nsl],
                                w2_sb[:, 2 * kp : 2 * kp + 2, m * 128 : (m + 1) * 128],
                                h1T[:, 2 * kp : 2 * kp + 2, nsl],
                                start=(kp == 0),
                                stop=(kp == MF // 2 - 1),
                                perf_mode=DR,
                            )
                    nc.scalar.activation(
                        h28[:, m, :], ps[:], AF.Relu, bias=b2_sb[:, m : m + 1]
                    )

                # ---- attention hidden: psa = (8h2) @ (2 Wa1), tanh ----
                aT = aT_pool.tile([128, MA, CHUNK], FP8)
                for ma in range(MA):
                    ps_a = psum_work.tile([128, CHUNK], F32, tag="ps")
                    for nh in range(NH):
                        nsl = slice(nh * NMM, (nh + 1) * NMM)
                        for kp in range(MF // 2):
                            nc.tensor.matmul(
                                ps_a[:, nsl],
                                wa1_sb[:, 2 * kp : 2 * kp + 2, ma * 128 : (ma + 1) * 128],
                                h28[:, 2 * kp : 2 * kp + 2, nsl],
                                start=(kp == 0),
                                stop=(kp == MF // 2 - 1),
                                perf_mode=DR,
                            )
                    nc.scalar.activation(
                        aT[:, ma, :], ps_a[:], AF.Tanh,
                        bias=ba1_sb[:, ma : ma + 1], scale=1.0 / 16,
                    )

                # ---- scores + padding penalty ----
                ps_r = psum_r_pool.tile([16, CHUNK], F32)
                for nh in range(NH):
                    nsl = slice(nh * NMM, (nh + 1) * NMM)
                    nc.tensor.matmul(
                        ps_r[:, nsl], wa2_sb[:], aT[:, 0:MA, nsl],
                        start=True, stop=False, perf_mode=DR,
                    )
                    nc.tensor.matmul(
                        ps_r[:, nsl], one_sb[:],
                        pen_sb[:, np0 + nh * NMM : np0 + (nh + 1) * NMM],
                        start=False, stop=True, skip_group_check=True,
                    )
                e_row = e_pool.tile([1, CHUNK], BF16)
                nc.scalar.activation(
                    e_row[:], ps_r[0:1, :], AF.Exp, scale=1.0 / 16
                )
                nc.sync.dma_start(e_out[:, np0 : np0 + CHUNK], e_row[:])

                # broadcast e across partitions
                eb_sb = eb_pool.tile([128, CHUNK], BF16)
                nc.gpsimd.partition_broadcast(eb_sb[:], e_row[:])

                # ---- pooling: fused multiply+free-axis accumulate per
                # 256-patch subtile: accum = sum(h28 * eb) ----
                dump = dump_pool.tile([128, SUBT], BF16, tag="dump")
                for m in range(MF):
                    for sub in range(SUBS):
                        t = c * SUBS + sub
                        ssl = slice(sub * SUBT, (sub + 1) * SUBT)
                        nc.vector.scalar_tensor_tensor(
                            dump[:],
                            h28[:, m, ssl],
                            0.0,
                            eb_sb[:, ssl],
                            OP.bypass,
                            OP.mult,
                            accum_out=subs_sb[:, m, t : t + 1],
                        )

            rep_ctx = (
                tc.For_i(0, V_REPEAT) if V_REPEAT > 1
                else contextlib.nullcontext()
            )
            with rep_ctx:
                xts = {cc: emit_xT(cc) for cc in range(min(2, C))}
                for c in range(C):
                    chunk_body(c, xts)

            nc.sync.dma_start(subs_out[:], subs_sb[:])

    nc.compile()
    _build_cache[key] = nc
    return nc


def _layout(bag_sizes, n_patches: int):
    """Per-core padded layout. Each bag padded up to a SUBT multiple, each
    core padded up to a CHUNK multiple.  Returns (np_pad, per-core list of
    [(bag_local_idx, src_lo, src_hi, dst_off), ...])."""
    sizes = _effective_sizes(bag_sizes, n_patches)
    offs = np.concatenate([[0], np.cumsum(sizes)])
    cores = []
    np_max = 0
    for i in range(N_CORES):
        pos = 0
        spans = []
        for bb in range(BAGS_PER_CORE):
            g = i * BAGS_PER_CORE + bb
            lo, hi = int(offs[g]), int(offs[g + 1])
            spans.append((bb, lo, hi, pos))
            pos += -(-max(hi - lo, 0) // SUBT) * SUBT
        cores.append(spans)
        np_max = max(np_max, pos)
    np_pad = max(int(-(-np_max // CHUNK) * CHUNK), CHUNK)
    return np_pad, cores


def _effective_sizes(bag_sizes, n_patches: int) -> np.ndarray:
    """Bag sizes matching jnp.repeat(arange(B), sizes, total_repeat_length=N):
    truncate if the sizes over-cover N, pad with the last bag if short."""
    sizes = np.asarray(bag_sizes).astype(np.int64).ravel()
    assert sizes.shape[0] == N_BAGS
    if int(sizes.sum()) == n_patches:
        return sizes
    reps = np.repeat(np.arange(N_BAGS), sizes)[:n_patches]
    if reps.shape[0] < n_patches:
        reps = np.concatenate(
            [reps, np.full(n_patches - reps.shape[0], N_BAGS - 1, np.int64)]
        )
    return np.bincount(reps, minlength=N_BAGS).astype(np.int64)


def _prepare(features, bag_sizes, W1, b1, W2, b2, Wa1, ba1, Wa2, ba2):
    """Host-side sharding + layout marshalling.
    Returns (np_pad, cores_spans, in_maps)."""
    features = np.ascontiguousarray(np.asarray(features, dtype=np.float32))
    np_pad, cores = _layout(bag_sizes, features.shape[0])
    KI = D_IN // 128

    # replicated weights, host-marshalled into device layouts (fp8 + scales)
    w1h = np.ascontiguousarray(
        (np.asarray(W1, np.float32) * 4.0)
        .reshape(KI, 128, D_FEAT).transpose(1, 0, 2)
    ).astype(NP_FP8)
    w2h = np.ascontiguousarray(
        (np.asarray(W2, np.float32) * 2.0)
        .reshape(D_FEAT // 128, 128, D_FEAT).transpose(1, 0, 2)
    ).astype(NP_FP8)
    wa1h = np.ascontiguousarray(
        (np.asarray(Wa1, np.float32) * 2.0)
        .reshape(D_FEAT // 128, 128, D_ATTN).transpose(1, 0, 2)
    ).astype(NP_FP8)
    wa2h = np.zeros((128, D_ATTN // 128, 16), NP_FP8)
    wa2h[:, :, 0] = (
        (np.asarray(Wa2, np.float32) * 16.0).reshape(D_ATTN // 128, 128).T
    ).astype(NP_FP8)

    feat8 = features.astype(NP_FP8)

    # h1 mean-centering: mu (in 4*h1 units) from a strided patch subsample,
    # computed with the same quantized X/W1 the device uses.  Centering h1
    # kills the coherent E[h1] @ (W2 - fp8(W2)) error term; the exact
    # mu @ W2 correction is folded into the GEMM2 bias.
    b1f = np.asarray(b1, np.float32).ravel()
    b2f = np.asarray(b2, np.float32).ravel()
    n_pat = features.shape[0]
    idx = np.arange(0, n_pat, max(1, n_pat // 2048))[:2048]
    W1q = (np.asarray(W1, np.float32) * 4.0).astype(NP_FP8).astype(np.float32)
    h1s = np.maximum(feat8[idx].astype(np.float32) @ W1q + 4.0 * b1f, 0.0)
    mu4 = h1s.mean(axis=0)  # [D_FEAT], in 4*h1 units

    nb1h = np.ascontiguousarray((-4.0 * b1f).reshape(-1, 128).T)
    a1h = np.ascontiguousarray((4.0 * b1f - mu4).reshape(-1, 128).T)
    b2corr = 8.0 * b2f + 2.0 * (
        mu4.astype(np.float64) @ np.asarray(W2, np.float64)
    ).astype(np.float32)
    b2h = np.ascontiguousarray(b2corr.reshape(-1, 128).T.astype(np.float32))
    ba1h = np.ascontiguousarray(np.asarray(ba1, np.float32).reshape(-1, 128).T)
    one16 = np.zeros((1, 16), NP_BF16)
    one16[0, 0] = 1.0

    in_maps = []
    for i in range(N_CORES):
        xpad = np.zeros((np_pad, D_IN), dtype=NP_FP8)
        penh = np.full((1, np_pad), -25600.0, dtype=NP_BF16)
        for bb, lo, hi, dst in cores[i]:
            n = hi - lo
            if n > 0:
                xpad[dst : dst + n] = feat8[lo:hi]
                penh[0, dst : dst + n] = 0.0
        xbh = np.ascontiguousarray(
            xpad.reshape(np_pad, KI, 128).transpose(2, 1, 0)
        )
        in_maps.append(
            {
                "xb": xbh,
                "w1": w1h,
                "w2": w2h,
                "wa1": wa1h,
                "wa2": wa2h,
                "nb1": nb1h,
                "a1": a1h,
                "b2": b2h,
                "ba1": ba1h,
                "pen": penh,
                "one16": one16,
            }
        )
    return np_pad, cores, in_maps


def run(inputs: dict, trace: bool = False):
    """Run on 8 cores. Returns (output[32,2] f32, BassKernelResults)."""
    np_pad, cores, in_maps = _prepare(
        inputs["features"],
        inputs["bag_sizes"],
        inputs["W1"],
        inputs["b1"],
        inputs["W2"],
        inputs["b2"],
        inputs["Wa1"],
        inputs["ba1"],
        inputs["Wa2"],
        inputs["ba2"],
    )
    nc = _build(np_pad)
    res = run_bass_kernel_spmd(
        nc, in_maps, core_ids=list(range(N_CORES)), trace=trace
    )

    acc = np.zeros((N_BAGS, D_FEAT))
    ssum = np.zeros((N_BAGS, 1))
    for i in range(N_CORES):
        # subs[p, m, t]: f = m*128 + p, per-subtile sums of 8*e*h2
        subs = np.asarray(res.results[i]["subs"], np.float64)
        e_i = np.asarray(res.results[i]["e_out"], np.float64)[0]  # [np_pad]
        for bb, lo, hi, dst in cores[i]:
            g = i * BAGS_PER_CORE + bb
            n = hi - lo
            t0 = dst // SUBT
            t1 = t0 + (-(-n // SUBT) if n > 0 else 0)
            bagT = subs[:, :, t0:t1].sum(axis=2)  # [128, MF]
            acc[g] = bagT.T.reshape(D_FEAT)
            ssum[g, 0] = e_i[dst : dst + n].sum()
    bag_emb = acc / (8.0 * ssum)
    out = bag_emb @ np.asarray(inputs["Wc"], np.float64) + np.asarray(
        inputs["bc"], np.float64
    )
    return out.astype(np.float32), res


def kernel(**inputs) -> np.ndarray:
    out, _ = run(inputs, trace=False)
    return out


# revision 17
# speedup vs baseline: 1.1361x; 1.1361x over previous
"""AttentionMIL on 8 TRN2 NeuronCores (Bass/Tile), data-parallel by bags,
with all GEMMs in fp8(e4m3) DoubleRow mode (2 k-subtiles contracted per
pass -> ~2x TensorE throughput vs bf16).

Model (per reference):
    h  = relu(relu(X @ W1 + b1) @ W2 + b2)            # [N, 512]
    s  = tanh(h @ Wa1 + ba1) @ Wa2 + ba2              # [N]
    w  = segment_softmax(s, bags)                     # per-bag softmax
    bag_emb = segment_sum(w * h)                      # [B, 512]
    out = bag_emb @ Wc + bc                           # [B, 2]

Distribution: 32 bags -> 8 cores x 4 bags, device-local reductions, no
collectives.  Each core's patches are laid out with every bag padded to a
multiple of SUBT(=256), the whole core padded to a multiple of CHUNK
(=1024), so every 256-patch pooling subtile lies in exactly one bag.

The whole chain runs in transposed space (features on SBUF partitions,
patches on the free axis); weights are the stationary operands.  X is
pre-transposed and cast to fp8 on the host: xb[p, kk, n] = X[n, 128*kk+p].

fp8 scaling (e4m3: max 240, min normal 2^-6):
    w1 = 4*W1, w2 = 2*W2, wa1 = 2*Wa1, wa2 = 16*Wa2 (all fp8)
    ps1 = X8 @ (4 W1)         -> h1 := fp8(relu(ps1 + 4 b1) - mu) (DVE)
    ps2 = h1 @ (2 W2)         -> h28 := fp8(relu(ps2 + bias2))  = 8*h2
    psa = (8h2) @ (2 Wa1)     -> aT := fp8(tanh(psa/16 + ba1))
    psr = aT @ (16 Wa2) + pen -> e := bf16(exp(psr/16))
mu is an E[4*h1] estimate from a host-side subsample; centering h1 kills
the coherent E[h1] @ (W2 - fp8(W2)) quantization error, and the exact
mu @ W2 term is folded into bias2 = 8*b2 + 2*mu@W2.  pen = -25600 on
padding rows (so e == 0 there; replaces a mask; ba2 is dropped -- softmax
is shift-invariant).  Per-bag max subtraction is dropped: tanh bounds
|s| < 17 so exp cannot overflow.

Pooling: e broadcast across partitions via GpSimd partition_broadcast,
then per 256-patch subtile one fused VectorE scalar_tensor_tensor
(product 8*e*h2 with free-axis accumulate) into subs[p, m, t].  The host
sums each bag's subtile columns (f64), divides by 8*sum(e), and applies
the tiny classifier: out = bag_emb @ Wc + bc.
"""

import numpy as np
import ml_dtypes

import concourse.tile as tile
from concourse import bacc, mybir
from concourse.bass_utils import run_bass_kernel_spmd

N_CORES = 8
N_BAGS = 32
BAGS_PER_CORE = N_BAGS // N_CORES  # 4
D_IN = 1024
D_FEAT = 512
D_ATTN = 256
CHUNK = 1024  # patches per pipeline chunk
SUBT = 256    # pooling subtile (bag padding granularity)
NMM = 512     # matmul moving free dim

BF16 = mybir.dt.bfloat16
F32 = mybir.dt.float32
FP8 = mybir.dt.float8e4
DR = mybir.MatmulPerfMode.DoubleRow
NP_FP8 = ml_dtypes.float8_e4m3  # TRN FP8_EXP4-compatible
NP_BF16 = ml_dtypes.bfloat16

_build_cache: dict = {}

V_REPEAT = 1  # For_i repetitions of the whole body (timing harness only)


def _build(np_pad: int) -> "bacc.Bacc":
    """Build + compile the per-core program for NP_PAD padded patches."""
    key = (np_pad, V_REPEAT)
    if key in _build_cache:
        return _build_cache[key]

    assert np_pad % CHUNK == 0
    C = np_pad // CHUNK          # number of chunks
    T = np_pad // SUBT           # number of pooling subtiles
    KI = D_IN // 128             # 8  k-subtiles for GEMM1
    MF = D_FEAT // 128           # 4  feature subtiles
    MA = D_ATTN // 128           # 2  attention subtiles
    SUBS = CHUNK // SUBT         # 4  pooling subtiles per chunk
    NH = CHUNK // NMM            # 2  matmul n-halves per chunk

    nc = bacc.Bacc("TRN2", target_bir_lowering=False, debug=False)

    xb = nc.dram_tensor("xb", [128, KI, np_pad], FP8, kind="ExternalInput").ap()
    w1 = nc.dram_tensor("w1", [128, KI, D_FEAT], FP8, kind="ExternalInput").ap()
    w2 = nc.dram_tensor("w2", [128, MF, D_FEAT], FP8, kind="ExternalInput").ap()
    wa1 = nc.dram_tensor("wa1", [128, MF, D_ATTN], FP8, kind="ExternalInput").ap()
    wa2 = nc.dram_tensor("wa2", [128, MA, 16], FP8, kind="ExternalInput").ap()
    nb1 = nc.dram_tensor("nb1", [128, MF], F32, kind="ExternalInput").ap()
    a1 = nc.dram_tensor("a1", [128, MF], F32, kind="ExternalInput").ap()
    b2 = nc.dram_tensor("b2", [128, MF], F32, kind="ExternalInput").ap()
    ba1 = nc.dram_tensor("ba1", [128, MA], F32, kind="ExternalInput").ap()
    pen = nc.dram_tensor("pen", [1, np_pad], BF16, kind="ExternalInput").ap()
    one16 = nc.dram_tensor("one16", [1, 16], BF16, kind="ExternalInput").ap()

    # per-subtile pooled sums: subs[p, m, t] = sum_{n in subtile t} 8*e*h2
    subs_out = nc.dram_tensor(
        "subs", [128, MF, T], F32, kind="ExternalOutput"
    ).ap()
    # e values (bf16, exactly as used in the pooling), padding rows -> 0
    e_out = nc.dram_tensor("e_out", [1, np_pad], BF16, kind="ExternalOutput").ap()

    AF = mybir.ActivationFunctionType
    OP = mybir.AluOpType

    import contextlib

    with tile.TileContext(nc) as tc:
        with (
            tc.tile_pool(name="const", bufs=1) as const,
            tc.tile_pool(name="xT", bufs=4) as xT_pool,
            tc.tile_pool(name="h1T", bufs=3) as h1T_pool,
            tc.tile_pool(name="h28", bufs=3) as h28_pool,
            tc.tile_pool(name="aT", bufs=3) as aT_pool,
            tc.tile_pool(name="erow", bufs=4) as e_pool,
            tc.tile_pool(name="ebc", bufs=3) as eb_pool,
            tc.tile_pool(name="dump", bufs=2) as dump_pool,
            tc.tile_pool(name="psw", bufs=3, space="PSUM") as psum_work,
            tc.tile_pool(name="psr", bufs=1, space="PSUM") as psum_r_pool,
        ):
            # ---- load constants ----
            w1_sb = const.tile([128, KI, D_FEAT], FP8)
            nc.sync.dma_start(w1_sb[:], w1[:])
            nb1_sb = const.tile([128, MF], F32)
            nc.sync.dma_start(nb1_sb[:], nb1[:])
            a1_sb = const.tile([128, MF], F32)
            nc.sync.dma_start(a1_sb[:], a1[:])
            w2_sb = const.tile([128, MF, D_FEAT], FP8)
            nc.sync.dma_start(w2_sb[:], w2[:])
            wa1_sb = const.tile([128, MF, D_ATTN], FP8)
            nc.sync.dma_start(wa1_sb[:], wa1[:])
            wa2_sb = const.tile([128, MA, 16], FP8)
            nc.sync.dma_start(wa2_sb[:], wa2[:])
            b2_sb = const.tile([128, MF], F32)
            nc.sync.dma_start(b2_sb[:], b2[:])
            ba1_sb = const.tile([128, MA], F32)
            nc.sync.dma_start(ba1_sb[:], ba1[:])
            pen_sb = const.tile([1, np_pad], BF16)
            nc.sync.dma_start(pen_sb[:], pen[:])
            one_sb = const.tile([1, 16], BF16)
            nc.sync.dma_start(one_sb[:], one16[:])

            subs_sb = const.tile([128, MF, T], F32)

            def emit_xT(cc):
                xt = xT_pool.tile([128, KI, CHUNK], FP8)
                nc.sync.dma_start(
                    xt[:], xb[:, :, cc * CHUNK : (cc + 1) * CHUNK]
                )
                return xt

            def chunk_body(c, xts):
                np0 = c * CHUNK

                xT = xts.pop(c)
                if c + 2 < C:
                    xts[c + 2] = emit_xT(c + 2)

                # ---- GEMM1: h1T = fp8(relu(W1^T X^T + 4 b1) - mu), DVE:
                # relu(x+b)-m = max(x, -b) + (b-m) -> one tensor_scalar ----
                # kp outer / n-half inner: each LDWEIGHTS serves both
                # 512-wide moving halves back-to-back
                h1T = h1T_pool.tile([128, MF, CHUNK], FP8)
                for m in range(MF):
                    ps = psum_work.tile([128, CHUNK], F32, tag="ps")
                    for kp in range(KI // 2):
                        for nh in range(NH):
                            nsl = slice(nh * NMM, (nh + 1) * NMM)
                            nc.tensor.matmul(
                                ps[:, nsl],
                                w1_sb[:, 2 * kp : 2 * kp + 2, m * 128 : (m + 1) * 128],
                                xT[:, 2 * kp : 2 * kp + 2, nsl],
                                start=(kp == 0),
                                stop=(kp == KI // 2 - 1),
                                perf_mode=DR,
                                skip_group_check=True,
                            )
                    nc.vector.tensor_scalar(
                        h1T[:, m, :], ps[:], nb1_sb[:, m : m + 1],
                        a1_sb[:, m : m + 1], OP.max, OP.add,
                    )

                # ---- GEMM2: h28 = fp8(relu(. + bias2)) on ScalarE ----
                h28 = h28_pool.tile([128, MF, CHUNK], FP8)
                for m in range(MF):
                    ps = psum_work.tile([128, CHUNK], F32, tag="ps")
                    for kp in range(MF // 2):
                        for nh in range(NH):
                            nsl = slice(nh * NMM, (nh + 1) * NMM)
                            nc.tensor.matmul(
                                ps[:, nsl],
                                w2_sb[:, 2 * kp : 2 * kp + 2, m * 128 : (m + 1) * 128],
                                h1T[:, 2 * kp : 2 * kp + 2, nsl],
                                start=(kp == 0),
                                stop=(kp == MF // 2 - 1),
                                perf_mode=DR,
                                skip_group_check=True,
                            )
                    nc.scalar.activation(
                        h28[:, m, :], ps[:], AF.Relu, bias=b2_sb[:, m : m + 1]
                    )

                # ---- attention hidden: psa = (8h2) @ (2 Wa1), tanh ----
                aT = aT_pool.tile([128, MA, CHUNK], FP8)
                for ma in range(MA):
                    ps_a = psum_work.tile([128, CHUNK], F32, tag="ps")
                    for kp in range(MF // 2):
                        for nh in range(NH):
                            nsl = slice(nh * NMM, (nh + 1) * NMM)
                            nc.tensor.matmul(
                                ps_a[:, nsl],
                                wa1_sb[:, 2 * kp : 2 * kp + 2, ma * 128 : (ma + 1) * 128],
                                h28[:, 2 * kp : 2 * kp + 2, nsl],
                                start=(kp == 0),
                                stop=(kp == MF // 2 - 1),
                                perf_mode=DR,
                                skip_group_check=True,
                            )
                    nc.scalar.activation(
                        aT[:, ma, :], ps_a[:], AF.Tanh,
                        bias=ba1_sb[:, ma : ma + 1], scale=1.0 / 16,
                    )

                # ---- scores + padding penalty ----
                ps_r = psum_r_pool.tile([16, CHUNK], F32)
                for nh in range(NH):
                    nsl = slice(nh * NMM, (nh + 1) * NMM)
                    nc.tensor.matmul(
                        ps_r[:, nsl], wa2_sb[:], aT[:, 0:MA, nsl],
                        start=True, stop=False, perf_mode=DR,
                    )
                    nc.tensor.matmul(
                        ps_r[:, nsl], one_sb[:],
                        pen_sb[:, np0 + nh * NMM : np0 + (nh + 1) * NMM],
                        start=False, stop=True, skip_group_check=True,
                    )
                e_row = e_pool.tile([1, CHUNK], BF16)
                nc.scalar.activation(
                    e_row[:], ps_r[0:1, :], AF.Exp, scale=1.0 / 16
                )
                nc.sync.dma_start(e_out[:, np0 : np0 + CHUNK], e_row[:])

                # broadcast e across partitions
                eb_sb = eb_pool.tile([128, CHUNK], BF16)
                nc.gpsimd.partition_broadcast(eb_sb[:], e_row[:])

                # ---- pooling: fused multiply+free-axis accumulate per
                # 256-patch subtile: accum = sum(h28 * eb) on VectorE
                # (GpSimd can't run TensorScalarPtr -- codegen rejects) ----
                dump = dump_pool.tile([128, SUBT], BF16, tag="dump")
                for m in range(MF):
                    for sub in range(SUBS):
                        t = c * SUBS + sub
                        ssl = slice(sub * SUBT, (sub + 1) * SUBT)
                        nc.vector.scalar_tensor_tensor(
                            dump[:],
                            h28[:, m, ssl],
                            0.0,
                            eb_sb[:, ssl],
                            OP.bypass,
                            OP.mult,
                            accum_out=subs_sb[:, m, t : t + 1],
                        )

            rep_ctx = (
                tc.For_i(0, V_REPEAT) if V_REPEAT > 1
                else contextlib.nullcontext()
            )
            with rep_ctx:
                xts = {cc: emit_xT(cc) for cc in range(min(2, C))}
                for c in range(C):
                    chunk_body(c, xts)

            nc.sync.dma_start(subs_out[:], subs_sb[:])

    nc.compile()
    _build_cache[key] = nc
    return nc


def _layout(bag_sizes, n_patches: int):
    """Per-core padded layout. Each bag padded up to a SUBT multiple, each
    core padded up to a CHUNK multiple.  Returns (np_pad, per-core list of
    [(bag_local_idx, src_lo, src_hi, dst_off), ...])."""
    sizes = _effective_sizes(bag_sizes, n_patches)
    offs = np.concatenate([[0], np.cumsum(sizes)])
    cores = []
    np_max = 0
    for i in range(N_CORES):
        pos = 0
        spans = []
        for bb in range(BAGS_PER_CORE):
            g = i * BAGS_PER_CORE + bb
            lo, hi = int(offs[g]), int(offs[g + 1])
            spans.append((bb, lo, hi, pos))
            pos += -(-max(hi - lo, 0) // SUBT) * SUBT
        cores.append(spans)
        np_max = max(np_max, pos)
    np_pad = max(int(-(-np_max // CHUNK) * CHUNK), CHUNK)
    return np_pad, cores


def _effective_sizes(bag_sizes, n_patches: int) -> np.ndarray:
    """Bag sizes matching jnp.repeat(arange(B), sizes, total_repeat_length=N):
    truncate if the sizes over-cover N, pad with the last bag if short."""
    sizes = np.asarray(bag_sizes).astype(np.int64).ravel()
    assert sizes.shape[0] == N_BAGS
    if int(sizes.sum()) == n_patches:
        return sizes
    reps = np.repeat(np.arange(N_BAGS), sizes)[:n_patches]
    if reps.shape[0] < n_patches:
        reps = np.concatenate(
            [reps, np.full(n_patches - reps.shape[0], N_BAGS - 1, np.int64)]
        )
    return np.bincount(reps, minlength=N_BAGS).astype(np.int64)


def _prepare(features, bag_sizes, W1, b1, W2, b2, Wa1, ba1, Wa2, ba2):
    """Host-side sharding + layout marshalling.
    Returns (np_pad, cores_spans, in_maps)."""
    features = np.ascontiguousarray(np.asarray(features, dtype=np.float32))
    np_pad, cores = _layout(bag_sizes, features.shape[0])
    KI = D_IN // 128

    # replicated weights, host-marshalled into device layouts (fp8 + scales)
    w1h = np.ascontiguousarray(
        (np.asarray(W1, np.float32) * 4.0)
        .reshape(KI, 128, D_FEAT).transpose(1, 0, 2)
    ).astype(NP_FP8)
    w2h = np.ascontiguousarray(
        (np.asarray(W2, np.float32) * 2.0)
        .reshape(D_FEAT // 128, 128, D_FEAT).transpose(1, 0, 2)
    ).astype(NP_FP8)
    wa1h = np.ascontiguousarray(
        (np.asarray(Wa1, np.float32) * 2.0)
        .reshape(D_FEAT // 128, 128, D_ATTN).transpose(1, 0, 2)
    ).astype(NP_FP8)
    wa2h = np.zeros((128, D_ATTN // 128, 16), NP_FP8)
    wa2h[:, :, 0] = (
        (np.asarray(Wa2, np.float32) * 16.0).reshape(D_ATTN // 128, 128).T
    ).astype(NP_FP8)

    feat8 = features.astype(NP_FP8)

    # h1 mean-centering: mu (in 4*h1 units) from a strided patch subsample,
    # computed with the same quantized X/W1 the device uses.  Centering h1
    # kills the coherent E[h1] @ (W2 - fp8(W2)) error term; the exact
    # mu @ W2 correction is folded into the GEMM2 bias.
    b1f = np.asarray(b1, np.float32).ravel()
    b2f = np.asarray(b2, np.float32).ravel()
    n_pat = features.shape[0]
    idx = np.arange(0, n_pat, max(1, n_pat // 2048))[:2048]
    W1q = (np.asarray(W1, np.float32) * 4.0).astype(NP_FP8).astype(np.float32)
    h1s = np.maximum(feat8[idx].astype(np.float32) @ W1q + 4.0 * b1f, 0.0)
    mu4 = h1s.mean(axis=0)  # [D_FEAT], in 4*h1 units

    nb1h = np.ascontiguousarray((-4.0 * b1f).reshape(-1, 128).T)
    a1h = np.ascontiguousarray((4.0 * b1f - mu4).reshape(-1, 128).T)
    b2corr = 8.0 * b2f + 2.0 * (
        mu4.astype(np.float64) @ np.asarray(W2, np.float64)
    ).astype(np.float32)
    b2h = np.ascontiguousarray(b2corr.reshape(-1, 128).T.astype(np.float32))
    ba1h = np.ascontiguousarray(np.asarray(ba1, np.float32).reshape(-1, 128).T)
    one16 = np.zeros((1, 16), NP_BF16)
    one16[0, 0] = 1.0

    in_maps = []
    for i in range(N_CORES):
        xpad = np.zeros((np_pad, D_IN), dtype=NP_FP8)
        penh = np.full((1, np_pad), -25600.0, dtype=NP_BF16)
        for bb, lo, hi, dst in cores[i]:
            n = hi - lo
            if n > 0:
                xpad[dst : dst + n] = feat8[lo:hi]
                penh[0, dst : dst + n] = 0.0
        xbh = np.ascontiguousarray(
            xpad.reshape(np_pad, KI, 128).transpose(2, 1, 0)
        )
        in_maps.append(
            {
                "xb": xbh,
                "w1": w1h,
                "w2": w2h,
                "wa1": wa1h,
                "wa2": wa2h,
                "nb1": nb1h,
                "a1": a1h,
                "b2": b2h,
                "ba1": ba1h,
                "pen": penh,
                "one16": one16,
            }
        )
    return np_pad, cores, in_maps


def run(inputs: dict, trace: bool = False):
    """Run on 8 cores. Returns (output[32,2] f32, BassKernelResults)."""
    np_pad, cores, in_maps = _prepare(
        inputs["features"],
        inputs["bag_sizes"],
        inputs["W1"],
        inputs["b1"],
        inputs["W2"],
        inputs["b2"],
        inputs["Wa1"],
        inputs["ba1"],
        inputs["Wa2"],
        inputs["ba2"],
    )
    nc = _build(np_pad)
    res = run_bass_kernel_spmd(
        nc, in_maps, core_ids=list(range(N_CORES)), trace=trace
    )

    acc = np.zeros((N_BAGS, D_FEAT))
    ssum = np.zeros((N_BAGS, 1))
    for i in range(N_CORES):
        # subs[p, m, t]: f = m*128 + p, per-subtile sums of 8*e*h2
        subs = np.asarray(res.results[i]["subs"], np.float64)
        e_i = np.asarray(res.results[i]["e_out"], np.float64)[0]  # [np_pad]
        for bb, lo, hi, dst in cores[i]:
            g = i * BAGS_PER_CORE + bb
            n = hi - lo
            t0 = dst // SUBT
            t1 = t0 + (-(-n // SUBT) if n > 0 else 0)
            bagT = subs[:, :, t0:t1].sum(axis=2)  # [128, MF]
            acc[g] = bagT.T.reshape(D_FEAT)
            ssum[g, 0] = e_i[dst : dst + n].sum()
    bag_emb = acc / (8.0 * ssum)
    out = bag_emb @ np.asarray(inputs["Wc"], np.float64) + np.asarray(
        inputs["bc"], np.float64
    )
    return out.astype(np.float32), res


def kernel(**inputs) -> np.ndarray:
    out, _ = run(inputs, trace=False)
    return out
